# revision 2
# baseline (speedup 1.0000x reference)
"""Distributed causal attention kernel for one TRN2 chip (8 NeuronCores).

Problem: out = (softmax_causal((xWq)(xWk)^T / sqrt(dh)) (xWv)) Wout + b
  N=8192, D_IN=1024, D_HEAD=128, D_OUT=1024, fp32 I/O (bf16 compute).

Sharding (zig-zag for causal load balance): the sequence is split into
16 chunks of 512 rows; core c owns chunks c and 15-c, so every core has
the same causal attention area (17 blocks of 512x512).  Q stays local,
K/V shards are computed locally and AllGather'ed (bf16).

Layout: scores are computed transposed, St[j, i] = K Q^T, so that the
softmax-weighted PV matmul needs no transposes: O^T[dh, i] = V^T P^T via
lhsT = V (natural), rhs = exp(St).  Softmax skips the max-subtraction
(scores are ~N(0,1), |s| < ~7) and defers normalization: the row-sum is
accumulated with a ones-vector matmul, and the division happens after
the output projection (bias folded in exactly via a rank-1 matmul of
rowsum x b_out before the division).

SPMD uniformity: all cores run one program.  Of the 17 causal work
items per core, 9 are statically identical across cores; the remaining
8 select their (q-half, kv-block, mask) via DVE registers derived from
partition_id and dynamic `ds()` slices, with PV partials accumulated
into an SBUF accumulator by the vector engine.
"""

import os
import sys

import numpy as np

if "/opt/trn_rl_repo" not in sys.path:
    sys.path.insert(0, "/opt/trn_rl_repo")

import concourse.bass as bass
import concourse.mybir as mybir
import concourse.tile as tile
from concourse import bacc
from concourse.bass import ds

F32 = mybir.dt.float32
BF16 = mybir.dt.bfloat16
AF = mybir.ActivationFunctionType
ALU = mybir.AluOpType


def build_program(cores=8, n=8192, d_in=1024, d_out=1024, dh=128,
                  enable_asserts=False):
    nchunk = 2 * cores            # zig-zag chunks
    ch = n // nchunk              # rows per chunk (512)
    r = 2 * ch                    # rows per core (1024)
    kd = d_in // 128              # contraction chunks for projections
    sub = ch // 128               # 128-row sub-chunks per kv block
    it = ch // 128                # 128-row i-tiles per half
    scale = float(dh) ** -0.5
    sw = sub * ch                 # score tile width (free elems per item)
    mo = n // d_in if d_out >= 512 else 1  # unused; keep simple
    m_t = 512 if d_out >= 512 else d_out   # out-proj moving width
    mh = d_out // m_t

    nc = bacc.Bacc("TRN2", target_bir_lowering=False, debug=False,
                   num_devices=cores, enable_asserts=enable_asserts)

    xT = nc.dram_tensor("xT", [d_in, r], F32, kind="ExternalInput")
    w_qkv = nc.dram_tensor("w_qkv", [d_in, 3 * dh], F32, kind="ExternalInput")
    b_qkv = nc.dram_tensor("b_qkv", [1, 3 * dh], F32, kind="ExternalInput")
    w_out = nc.dram_tensor("w_out", [dh, d_out], F32, kind="ExternalInput")
    b_out = nc.dram_tensor("b_out", [1, d_out], F32, kind="ExternalInput")
    out = nc.dram_tensor("out", [r, d_out], F32, kind="ExternalOutput")

    with tile.TileContext(nc) as tc:
        with (
            tc.tile_pool(name="dram", bufs=1, space="DRAM") as dram,
            tc.tile_pool(name="consts", bufs=1) as consts,
            tc.tile_pool(name="params", bufs=1) as params,
            tc.tile_pool(name="qkv", bufs=1) as qkvp,
            tc.tile_pool(name="gath", bufs=1) as gath,
            tc.tile_pool(name="accs", bufs=1) as accs,
            tc.tile_pool(name="stage", bufs=3) as stagep,
            tc.tile_pool(name="exps", bufs=3) as exps,
            tc.tile_pool(name="dyn", bufs=3) as dynp,
            tc.tile_pool(name="epi", bufs=2) as epip,
            tc.tile_pool(name="outp", bufs=3) as outpp,
            tc.tile_pool(name="st_ps", bufs=1, space="PSUM") as st_ps,
            tc.tile_pool(name="o1_ps", bufs=1, space="PSUM") as o1_ps,
            tc.tile_pool(name="rs1_ps", bufs=1, space="PSUM") as rs1_ps,
            tc.tile_pool(name="misc_ps", bufs=2, space="PSUM") as misc_ps,
        ):
            # ---------------- constants ----------------
            ones_col = consts.tile([128, 1], BF16, tag="ones_col")
            nc.gpsimd.memset(ones_col[:], 1.0)
            ones_row = consts.tile([1, max(ch, 128)], BF16, tag="ones_row")
            nc.gpsimd.memset(ones_row[:], 1.0)
            one_f = consts.tile([1, 1], F32, tag="one_f")
            nc.gpsimd.memset(one_f[:], 1.0)
            # mask strips: [:, 0:sw] = ones, [:, sw:2*sw] = causal triangles
            masks = consts.tile([128, 2 * sw], BF16, tag="masks")
            nc.gpsimd.memset(masks[:], 1.0)
            for u in range(sub):
                # visible (keep 1.0) iff i_local >= 128*u + j_local
                nc.gpsimd.affine_select(
                    out=masks[:, sw + u * ch: sw + (u + 1) * ch],
                    in_=masks[:, sw + u * ch: sw + (u + 1) * ch],
                    compare_op=ALU.is_ge,
                    fill=0.0,
                    base=-(128 * u),
                    pattern=[[1, ch]],
                    channel_multiplier=-1,
                )

            # ---------------- params: load + cast to bf16 ----------------
            xT_bf = params.tile([128, kd, r], BF16, tag="xT_bf")
            for k in range(kd):
                st = stagep.tile([128, r], F32, tag="stage_f32")
                nc.sync.dma_start(st[:], xT[128 * k:128 * (k + 1), :])
                nc.vector.tensor_copy(xT_bf[:, k, :], st[:])
            wqkv_bf = params.tile([128, kd, 3 * dh], BF16, tag="wqkv_bf")
            for k in range(kd):
                st = stagep.tile([128, 3 * dh], F32, tag="stage_w")
                nc.sync.dma_start(st[:], w_qkv[128 * k:128 * (k + 1), :])
                nc.vector.tensor_copy(wqkv_bf[:, k, :], st[:])
            wout_bf = params.tile([dh, d_out], BF16, tag="wout_bf")
            st = stagep.tile([dh, d_out], F32, tag="stage_w")
            nc.sync.dma_start(st[:dh, :d_out], w_out[:, :])
            nc.vector.tensor_copy(wout_bf[:], st[:dh, :d_out])
            bqkv_bf = params.tile([1, 3 * dh], BF16, tag="bqkv_bf")
            st = stagep.tile([1, 3 * dh], F32, tag="stage_b")
            nc.sync.dma_start(st[:], b_qkv[:, :])
            nc.vector.tensor_copy(bqkv_bf[:], st[:])
            bout_bf = params.tile([1, d_out], BF16, tag="bout_bf")
            st = stagep.tile([1, d_out], F32, tag="stage_b2")
            nc.sync.dma_start(st[:], b_out[:, :])
            nc.vector.tensor_copy(bout_bf[:], st[:])

            # ---------------- qkv projection (local shard) ----------------
            # qT/kT transposed [dh, i]; v natural [i, dh]
            qT_bf = qkvp.tile([128, r], BF16, tag="qT_bf")
            kT_loc = qkvp.tile([128, r], BF16, tag="kT_loc")
            v_loc = qkvp.tile([128, r // 128, dh], BF16, tag="v_loc")
            for h in range(2):
                for nt in range(2):  # 0=q, 1=k
                    ps = misc_ps.tile([128, ch], F32, tag="mps")
                    for k in range(kd):
                        nc.tensor.matmul(
                            ps[:],
                            lhsT=wqkv_bf[:, k, nt * dh:(nt + 1) * dh],
                            rhs=xT_bf[:, k, h * ch:(h + 1) * ch],
                            start=(k == 0), stop=False)
                    nc.tensor.matmul(
                        ps[:],
                        lhsT=bqkv_bf[0:1, nt * dh:(nt + 1) * dh],
                        rhs=ones_row[0:1, 0:ch],
                        start=False, stop=True)
                    dest = qT_bf if nt == 0 else kT_loc
                    nc.vector.tensor_copy(dest[:, h * ch:(h + 1) * ch], ps[:])
            for t in range(r // 128):
                ps = misc_ps.tile([128, dh], F32, tag="mps")
                for k in range(kd):
                    nc.tensor.matmul(
                        ps[:],
                        lhsT=xT_bf[:, k, 128 * t:128 * (t + 1)],
                        rhs=wqkv_bf[:, k, 2 * dh:3 * dh],
                        start=(k == 0), stop=False)
                nc.tensor.matmul(
                    ps[:],
                    lhsT=ones_row[0:1, 0:128],
                    rhs=bqkv_bf[0:1, 2 * dh:3 * dh],
                    start=False, stop=True)
                nc.vector.tensor_copy(v_loc[:, t, :], ps[:])

            # ---------------- all-gather K/V ----------------
            kT_bounce = dram.tile([dh, r], BF16, tag="kT_bounce")
            v_bounce = dram.tile([r, dh], BF16, tag="v_bounce")
            kT_gath = dram.tile([cores * dh, r], BF16, tag="kT_gath")
            v_gath = dram.tile([n, dh], BF16, tag="v_gath")
            nc.sync.dma_start(kT_bounce[:], kT_loc[:])
            nc.sync.dma_start(
                v_bounce.rearrange("(a p) c -> p a c", p=128), v_loc[:])
            rg = [list(range(cores))]
            nc.gpsimd.collective_compute(
                "AllGather", ALU.bypass, replica_groups=rg,
                ins=[kT_bounce.opt()], outs=[kT_gath.opt()])
            nc.gpsimd.collective_compute(
                "AllGather", ALU.bypass, replica_groups=rg,
                ins=[v_bounce.opt()], outs=[v_gath.opt()])

            # stage gathered kv into SBUF, in global block order
            kT_all = gath.tile([128, nchunk, ch], BF16, tag="kT_all")
            v_all = gath.tile([128, nchunk, sub, dh], BF16, tag="v_all")
            for b in range(nchunk):
                rr, hh = (b, 0) if b < cores else (nchunk - 1 - b, 1)
                nc.sync.dma_start(
                    kT_all[:, b, :],
                    kT_gath[128 * rr:128 * (rr + 1), hh * ch:(hh + 1) * ch])
                nc.sync.dma_start(
                    v_all[:, b, :, :],
                    v_gath[r * rr + hh * ch: r * rr + (hh + 1) * ch, :]
                    .rearrange("(a p) c -> p a c", p=128))

            # ---------------- attention ----------------
            # per-core registers (vector engine only)
            c_reg = nc.vector.partition_id()

            O_acc = accs.tile([128, 2 * ch], F32, tag="O_acc")
            rs_acc = accs.tile([1, 2 * ch], F32, tag="rs_acc")
            nc.gpsimd.memset(O_acc[:], 0.0)
            nc.gpsimd.memset(rs_acc[:], 0.0)

            O1 = o1_ps.tile([128, ch], F32, tag="O1")
            rs1 = rs1_ps.tile([1, ch], F32, tag="rs1")

            n_static = cores + 1          # items 0..cores: half 1, block=t
            n_items = nchunk + 1

            def emit_item(k_ap_fn, v_ap_fn, q_ap, mask_dyn, o_ps, rs_ps,
                          o_startstop, mask_static=False):
                """k_ap_fn(u) -> [128,128] lhsT; v_ap_fn(u) -> [128,dh] lhsT."""
                stp = st_ps.tile([128, sw], F32, tag="St")
                for u in range(sub):
                    nc.tensor.matmul(
                        stp[:, u * ch:(u + 1) * ch],
                        lhsT=k_ap_fn(u), rhs=q_ap,
                        start=True, stop=True)
                ex = exps.tile([128, sw], BF16, tag="ex")
                nc.scalar.activation(ex[:], stp[:], AF.Exp, scale=scale)
                if mask_static:
                    nc.vector.tensor_mul(ex[:], ex[:], masks[:, sw:2 * sw])
                elif mask_dyn is not None:
                    nc.vector.tensor_mul(ex[:], ex[:], mask_dyn)
                o_start, o_stop = o_startstop
                for u in range(sub):
                    nc.tensor.matmul(
                        o_ps[:],
                        lhsT=v_ap_fn(u), rhs=ex[:, u * ch:(u + 1) * ch],
                        start=(o_start and u == 0),
                        stop=(o_stop and u == sub - 1))
                    nc.tensor.matmul(
                        rs_ps[0:1, :],
                        lhsT=ones_col[:, 0:1], rhs=ex[:, u * ch:(u + 1) * ch],
                        start=(o_start and u == 0),
                        stop=(o_stop and u == sub - 1))

            # static items: half 1 (chunk nchunk-1-c), blocks 0..cores
            for t in range(n_static):
                b = t
                mask_dyn = None
                if t == cores:
                    # diagonal iff c == cores-1
                    midx = nc.snap((9 + c_reg) >> 4, donate=True,
                                   min_val=0, max_val=1)
                    mask_dyn = masks[:, ds(midx * sw, sw)]
                emit_item(
                    lambda u, b=b: kT_all[:, b, 128 * u:128 * (u + 1)],
                    lambda u, b=b: v_all[:, b, u, :],
                    qT_bf[:, ch:2 * ch],
                    mask_dyn, O1, rs1,
                    (t == 0, t == n_static - 1))

            # dynamic items
            for t in range(n_static, n_items):
                isl = nc.snap(((nchunk + cores - 1 - t) - c_reg) >> 3,
                              donate=True, min_val=0, max_val=1)
                blk = nc.snap(t - nchunk + c_reg + isl * (nchunk - c_reg),
                              donate=True, min_val=0, max_val=nchunk - 1)
                p_d = nc.snap(c_reg + isl * (nchunk - 1 - 2 * c_reg),
                              donate=True, min_val=0, max_val=nchunk - 1)
                dd = nc.snap(blk - p_d + 16, donate=True,
                             min_val=1, max_val=16)
                midx = nc.snap(dd >> 4, donate=True, min_val=0, max_val=1)

                qst = dynp.tile([128, ch], BF16, tag="qst")
                nc.vector.tensor_copy(qst[:], qT_bf[:, ds(isl * ch, ch)])
                kst = dynp.tile([128, 1, ch], BF16, tag="kst")
                nc.vector.tensor_copy(kst[:], kT_all[:, ds(blk, 1), :])
                vst = dynp.tile([128, 1, sub, dh], BF16, tag="vst")
                nc.vector.tensor_copy(vst[:], v_all[:, ds(blk, 1), :, :])

                o_part = misc_ps.tile([128, ch], F32, tag="mps")
                rs_part = misc_ps.tile([1, ch], F32, tag="mps")
                emit_item(
                    lambda u: kst[:, 0, 128 * u:128 * (u + 1)],
                    lambda u: vst[:, 0, u, :],
                    qst[:],
                    masks[:, ds(midx * sw, sw)],
                    o_part, rs_part,
                    (True, True))
                nc.vector.tensor_add(
                    O_acc[:, ds(isl * ch, ch)],
                    O_acc[:, ds(isl * ch, ch)], o_part[:])
                nc.vector.tensor_add(
                    rs_acc[0:1, ds(isl * ch, ch)],
                    rs_acc[0:1, ds(isl * ch, ch)], rs_part[0:1, :])

            # ---------------- epilogue ----------------
            for h in range(2):
                Ot = epip.tile([128, ch], BF16, tag="Ot")
                rs_row = epip.tile([1, ch], F32, tag="rs_row")
                if h == 1:
                    nc.vector.tensor_add(Ot[:], O_acc[:, ch:2 * ch], O1[:])
                    nc.vector.tensor_add(rs_row[:], rs_acc[0:1, ch:2 * ch],
                                         rs1[0:1, :])
                else:
                    nc.vector.tensor_copy(Ot[:], O_acc[:, 0:ch])
                    nc.vector.tensor_copy(rs_row[:], rs_acc[0:1, 0:ch])
                rs_bf = epip.tile([1, ch], BF16, tag="rs_bf")
                nc.vector.tensor_copy(rs_bf[:], rs_row[:])
                for tt in range(it):
                    rsT = misc_ps.tile([128, 1], F32, tag="mps")
                    nc.tensor.matmul(
                        rsT[:],
                        lhsT=rs_row[0:1, 128 * tt:128 * (tt + 1)],
                        rhs=one_f[0:1, 0:1], start=True, stop=True)
                    rec = epip.tile([128, 1], F32, tag="rec")
                    nc.vector.reciprocal(rec[:], rsT[:])
                    for m in range(mh):
                        ops = misc_ps.tile([128, m_t], F32, tag="mps")
                        nc.tensor.matmul(
                            ops[:],
                            lhsT=Ot[:, 128 * tt:128 * (tt + 1)],
                            rhs=wout_bf[:, m * m_t:(m + 1) * m_t],
                            start=True, stop=False)
                        nc.tensor.matmul(
                            ops[:],
                            lhsT=rs_bf[0:1, 128 * tt:128 * (tt + 1)],
                            rhs=bout_bf[0:1, m * m_t:(m + 1) * m_t],
                            start=False, stop=True)
                        osb = outpp.tile([128, m_t], F32, tag="osb")
                        nc.vector.tensor_scalar_mul(osb[:], ops[:], rec[:, 0:1])
                        nc.sync.dma_start(
                            out[h * ch + 128 * tt: h * ch + 128 * (tt + 1),
                                m * m_t:(m + 1) * m_t],
                            osb[:])

    nc.compile()
    return nc


# ---------------- host side ----------------

_CACHED = {}


def _get_program(key, **kw):
    if key not in _CACHED:
        _CACHED[key] = build_program(**kw)
    return _CACHED[key]


def shard_inputs(x, w_qkv, b_qkv, w_out, b_out, cores=8):
    n = x.shape[0]
    nchunk = 2 * cores
    ch = n // nchunk
    in_maps = []
    for c in range(cores):
        xs = np.concatenate(
            [x[ch * c: ch * (c + 1)],
             x[ch * (nchunk - 1 - c): ch * (nchunk - c)]], axis=0)
        in_maps.append({
            "xT": np.ascontiguousarray(xs.T).astype(np.float32),
            "w_qkv": np.ascontiguousarray(w_qkv).astype(np.float32),
            "b_qkv": np.ascontiguousarray(
                b_qkv).reshape(1, -1).astype(np.float32),
            "w_out": np.ascontiguousarray(w_out).astype(np.float32),
            "b_out": np.ascontiguousarray(
                b_out).reshape(1, -1).astype(np.float32),
        })
    return in_maps


def unshard_output(results, n, d_out, cores=8):
    nchunk = 2 * cores
    ch = n // nchunk
    out = np.empty((n, d_out), dtype=np.float32)
    for c in range(cores):
        o = results[c]["out"]
        out[ch * c: ch * (c + 1)] = o[:ch]
        out[ch * (nchunk - 1 - c): ch * (nchunk - c)] = o[ch:]
    return out


def kernel(x, w_qkv, b_qkv, w_out, b_out):
    from concourse.bass_utils import run_bass_kernel_spmd

    x = np.asarray(x)
    w_qkv = np.asarray(w_qkv)
    b_qkv = np.asarray(b_qkv)
    w_out = np.asarray(w_out)
    b_out = np.asarray(b_out)
    cores = 8
    n, d_in = x.shape
    d_out = w_out.shape[1]
    dh = w_out.shape[0]
    nc = _get_program(
        (cores, n, d_in, d_out, dh),
        cores=cores, n=n, d_in=d_in, d_out=d_out, dh=dh)
    in_maps = shard_inputs(x, w_qkv, b_qkv, w_out, b_out, cores)
    res = run_bass_kernel_spmd(nc, in_maps, core_ids=list(range(cores)))
    return unshard_output(res.results, n, d_out, cores)


# revision 8
# speedup vs baseline: 1.0364x; 1.0364x over previous
"""Distributed causal attention kernel for one TRN2 chip (8 NeuronCores).

Problem: out = (softmax_causal((xWq)(xWk)^T / sqrt(dh)) (xWv)) Wout + b
  N=8192, D_IN=1024, D_HEAD=128, D_OUT=1024, fp32 I/O (bf16 compute).

Sharding (zig-zag for causal load balance): the sequence is split into
16 chunks of 512 rows; core c owns chunks c and 15-c, so every core has
the same causal attention area (17 blocks of 512x512).  Q stays local,
K/V shards are computed locally and AllGather'ed (bf16).

Layout: scores are computed transposed, St[j, i] = K Q^T, so that the
softmax-weighted PV matmul needs no transposes: O^T[dh, i] = V^T P^T via
lhsT = V (natural), rhs = exp(St).  Softmax skips the max-subtraction
(scores are ~N(0,1), |s| < ~7) and defers normalization: the row-sum is
accumulated with a ones-vector matmul, and the division happens after
the output projection (bias folded in exactly via a rank-1 matmul of
rowsum x b_out before the division).

SPMD uniformity: all cores run one program.  Of the 17 causal work
items per core, 9 are statically identical across cores; the remaining
8 select their (q-half, kv-block, mask) via DVE registers derived from
partition_id and dynamic `ds()` slices, with PV partials accumulated
into an SBUF accumulator by the vector engine.
"""

import os
import sys

import numpy as np

if "/opt/trn_rl_repo" not in sys.path:
    sys.path.insert(0, "/opt/trn_rl_repo")

import concourse.bass as bass
import concourse.mybir as mybir
import concourse.tile as tile
from concourse import bacc
from concourse.bass import ds

F32 = mybir.dt.float32
BF16 = mybir.dt.bfloat16
AF = mybir.ActivationFunctionType
ALU = mybir.AluOpType


def build_program(cores=8, n=8192, d_in=1024, d_out=1024, dh=128,
                  enable_asserts=False):
    nchunk = 2 * cores            # zig-zag chunks
    ch = n // nchunk              # rows per chunk (512)
    r = 2 * ch                    # rows per core (1024)
    kd = d_in // 128              # contraction chunks for projections
    sub = ch // 128               # 128-row sub-chunks per kv block
    it = ch // 128                # 128-row i-tiles per half
    scale = float(dh) ** -0.5
    sw = sub * ch                 # score tile width (free elems per item)
    mo = n // d_in if d_out >= 512 else 1  # unused; keep simple
    m_t = 512 if d_out >= 512 else d_out   # out-proj moving width
    mh = d_out // m_t

    nc = bacc.Bacc("TRN2", target_bir_lowering=False, debug=False,
                   num_devices=cores, enable_asserts=enable_asserts)

    xT = nc.dram_tensor("xT", [d_in, r], F32, kind="ExternalInput")
    w_qkv = nc.dram_tensor("w_qkv", [d_in, 3 * dh], F32, kind="ExternalInput")
    b_qkv = nc.dram_tensor("b_qkv", [1, 3 * dh], F32, kind="ExternalInput")
    w_out = nc.dram_tensor("w_out", [dh, d_out], F32, kind="ExternalInput")
    b_out = nc.dram_tensor("b_out", [1, d_out], F32, kind="ExternalInput")
    out = nc.dram_tensor("out", [r, d_out], F32, kind="ExternalOutput")

    with tile.TileContext(nc) as tc:
        with (
            tc.tile_pool(name="dram", bufs=1, space="DRAM") as dram,
            tc.tile_pool(name="consts", bufs=1) as consts,
            tc.tile_pool(name="params", bufs=1) as params,
            tc.tile_pool(name="qkv", bufs=1) as qkvp,
            tc.tile_pool(name="gath", bufs=1) as gath,
            tc.tile_pool(name="accs", bufs=1) as accs,
            tc.tile_pool(name="stage", bufs=3) as stagep,
            tc.tile_pool(name="exps", bufs=4) as exps,
            tc.tile_pool(name="dyn", bufs=3) as dynp,
            tc.tile_pool(name="epi", bufs=2) as epip,
            tc.tile_pool(name="outp", bufs=3) as outpp,
            tc.tile_pool(name="st_ps", bufs=2, space="PSUM") as st_ps,
            tc.tile_pool(name="o1_ps", bufs=1, space="PSUM") as o1_ps,
            tc.tile_pool(name="rs1_ps", bufs=1, space="PSUM") as rs1_ps,
            tc.tile_pool(name="misc_ps", bufs=2, space="PSUM") as misc_ps,
        ):
            # ---------------- constants ----------------
            ones_col = consts.tile([128, 1], BF16, tag="ones_col")
            nc.gpsimd.memset(ones_col[:], 1.0)
            ones_row = consts.tile([1, max(ch, 128)], BF16, tag="ones_row")
            nc.gpsimd.memset(ones_row[:], 1.0)
            one_f = consts.tile([1, 1], F32, tag="one_f")
            nc.gpsimd.memset(one_f[:], 1.0)
            # mask strips: [:, 0:sw] = ones, [:, sw:2*sw] = causal triangles
            masks = consts.tile([128, 2 * sw], BF16, tag="masks")
            nc.gpsimd.memset(masks[:], 1.0)
            for u in range(sub):
                # visible (keep 1.0) iff i_local >= 128*u + j_local
                nc.gpsimd.affine_select(
                    out=masks[:, sw + u * ch: sw + (u + 1) * ch],
                    in_=masks[:, sw + u * ch: sw + (u + 1) * ch],
                    compare_op=ALU.is_ge,
                    fill=0.0,
                    base=-(128 * u),
                    pattern=[[1, ch]],
                    channel_multiplier=-1,
                )

            # ---------------- params: load + cast to bf16 ----------------
            xT_bf = params.tile([128, kd, r], BF16, tag="xT_bf")
            for k in range(kd):
                st = stagep.tile([128, r], F32, tag="stage_f32")
                nc.sync.dma_start(st[:], xT[128 * k:128 * (k + 1), :])
                nc.vector.tensor_copy(xT_bf[:, k, :], st[:])
            wqkv_bf = params.tile([128, kd, 3 * dh], BF16, tag="wqkv_bf")
            for k in range(kd):
                st = stagep.tile([128, 3 * dh], F32, tag="stage_w")
                nc.sync.dma_start(st[:], w_qkv[128 * k:128 * (k + 1), :])
                nc.vector.tensor_copy(wqkv_bf[:, k, :], st[:])
            wout_bf = params.tile([dh, d_out], BF16, tag="wout_bf")
            st = stagep.tile([dh, d_out], F32, tag="stage_w")
            nc.sync.dma_start(st[:dh, :d_out], w_out[:, :])
            nc.vector.tensor_copy(wout_bf[:], st[:dh, :d_out])
            bqkv_bf = params.tile([1, 3 * dh], BF16, tag="bqkv_bf")
            st = stagep.tile([1, 3 * dh], F32, tag="stage_b")
            nc.sync.dma_start(st[:], b_qkv[:, :])
            nc.vector.tensor_copy(bqkv_bf[:], st[:])
            bout_bf = params.tile([1, d_out], BF16, tag="bout_bf")
            st = stagep.tile([1, d_out], F32, tag="stage_b2")
            nc.sync.dma_start(st[:], b_out[:, :])
            nc.vector.tensor_copy(bout_bf[:], st[:])

            # ---------------- qkv projection (local shard) ----------------
            # qT/kT transposed [dh, i]; v natural [i, dh]
            qT_bf = qkvp.tile([128, r], BF16, tag="qT_bf")
            kT_loc = qkvp.tile([128, r], BF16, tag="kT_loc")
            v_loc = qkvp.tile([128, r // 128, dh], BF16, tag="v_loc")
            for h in range(2):
                for nt in range(2):  # 0=q, 1=k
                    ps = misc_ps.tile([128, ch], F32, tag="mps")
                    for k in range(kd):
                        nc.tensor.matmul(
                            ps[:],
                            lhsT=wqkv_bf[:, k, nt * dh:(nt + 1) * dh],
                            rhs=xT_bf[:, k, h * ch:(h + 1) * ch],
                            start=(k == 0), stop=False)
                    nc.tensor.matmul(
                        ps[:],
                        lhsT=bqkv_bf[0:1, nt * dh:(nt + 1) * dh],
                        rhs=ones_row[0:1, 0:ch],
                        start=False, stop=True)
                    dest = qT_bf if nt == 0 else kT_loc
                    nc.vector.tensor_copy(dest[:, h * ch:(h + 1) * ch], ps[:])
            for t in range(r // 128):
                ps = misc_ps.tile([128, dh], F32, tag="mps")
                for k in range(kd):
                    nc.tensor.matmul(
                        ps[:],
                        lhsT=xT_bf[:, k, 128 * t:128 * (t + 1)],
                        rhs=wqkv_bf[:, k, 2 * dh:3 * dh],
                        start=(k == 0), stop=False)
                nc.tensor.matmul(
                    ps[:],
                    lhsT=ones_row[0:1, 0:128],
                    rhs=bqkv_bf[0:1, 2 * dh:3 * dh],
                    start=False, stop=True)
                nc.vector.tensor_copy(v_loc[:, t, :], ps[:])

            # ---------------- all-gather K/V (one combined collective) ----
            # bounce rows 0..127 = kT [dh, r]; rows 128..255 = v_loc as-is
            # ([128, r/128, dh] flattened along free).
            kv_bounce = dram.tile([2 * dh, r], BF16, tag="kv_bounce")
            kv_gath = dram.tile([cores * 2 * dh, r], BF16, tag="kv_gath")
            nc.sync.dma_start(kv_bounce[0:dh, :], kT_loc[:])
            nc.sync.dma_start(
                kv_bounce[dh:2 * dh, :].rearrange(
                    "p (t d) -> p t d", t=r // 128),
                v_loc[:])
            rg = [list(range(cores))]
            nc.gpsimd.collective_compute(
                "AllGather", ALU.bypass, replica_groups=rg,
                ins=[kv_bounce.opt()], outs=[kv_gath.opt()])

            # stage gathered kv into SBUF, in global block order.
            # core rr's kT is kv_gath rows [256rr, 256rr+128); its v rows
            # [256rr+128, 256rr+256): row 128+p col 128t+d = v[128t+p, d].
            kT_all = gath.tile([128, nchunk, ch], BF16, tag="kT_all")
            v_all = gath.tile([128, nchunk, sub, dh], BF16, tag="v_all")
            for b in range(nchunk):
                rr, hh = (b, 0) if b < cores else (nchunk - 1 - b, 1)
                base = 2 * dh * rr
                nc.sync.dma_start(
                    kT_all[:, b, :],
                    kv_gath[base:base + 128, hh * ch:(hh + 1) * ch])
                nc.sync.dma_start(
                    v_all[:, b, :, :],
                    kv_gath[base + 128:base + 256,
                            (hh * sub) * dh:(hh * sub + sub) * dh]
                    .rearrange("p (u d) -> p u d", u=sub))

            # ---------------- attention ----------------
            # per-core registers (vector engine only)
            c_reg = nc.vector.partition_id()

            O_acc = accs.tile([128, 2 * ch], F32, tag="O_acc")
            rs_acc = accs.tile([1, 2 * ch], F32, tag="rs_acc")
            nc.gpsimd.memset(O_acc[:], 0.0)
            nc.gpsimd.memset(rs_acc[:], 0.0)

            O1 = o1_ps.tile([128, ch], F32, tag="O1")
            rs1 = rs1_ps.tile([1, ch], F32, tag="rs1")

            n_static = cores + 1          # items 0..cores: half 1, block=t
            n_items = nchunk + 1

            gs = max(1, sub // 2)  # subchunks per St group (double-buffer)

            def emit_item(k_ap_fn, v_ap_fn, q_ap, midx_reg, o_ps, rs_ps,
                          o_startstop, mask_static=False):
                """k_ap_fn(u) -> [128,128] lhsT; v_ap_fn(u) -> [128,dh] lhsT."""
                o_start, o_stop = o_startstop
                for g in range(0, sub, gs):
                    us = range(g, min(sub, g + gs))
                    gw = len(us) * ch
                    stp = st_ps.tile([128, gs * ch], F32, tag="St")
                    for ui, u in enumerate(us):
                        nc.tensor.matmul(
                            stp[:, ui * ch:(ui + 1) * ch],
                            lhsT=k_ap_fn(u), rhs=q_ap,
                            start=True, stop=True)
                    ex = exps.tile([128, gs * ch], BF16, tag="ex")
                    nc.scalar.activation(ex[:, :gw], stp[:, :gw], AF.Exp,
                                         scale=scale)
                    if mask_static:
                        nc.vector.tensor_mul(
                            ex[:, :gw], ex[:, :gw],
                            masks[:, sw + g * ch: sw + g * ch + gw])
                    elif midx_reg is not None:
                        nc.vector.tensor_mul(
                            ex[:, :gw], ex[:, :gw],
                            masks[:, ds(midx_reg * sw + g * ch, gw)])
                    for ui, u in enumerate(us):
                        nc.tensor.matmul(
                            o_ps[:],
                            lhsT=v_ap_fn(u), rhs=ex[:, ui * ch:(ui + 1) * ch],
                            start=(o_start and u == 0),
                            stop=(o_stop and u == sub - 1))
                        nc.tensor.matmul(
                            rs_ps[0:1, :],
                            lhsT=ones_col[:, 0:1],
                            rhs=ex[:, ui * ch:(ui + 1) * ch],
                            start=(o_start and u == 0),
                            stop=(o_stop and u == sub - 1))

            # static items: half 1 (chunk nchunk-1-c), blocks 0..cores
            for t in range(n_static):
                b = t
                midx_reg = None
                if t == cores:
                    # diagonal iff c == cores-1
                    midx_reg = nc.snap((9 + c_reg) >> 4, donate=True,
                                       min_val=0, max_val=1)
                emit_item(
                    lambda u, b=b: kT_all[:, b, 128 * u:128 * (u + 1)],
                    lambda u, b=b: v_all[:, b, u, :],
                    qT_bf[:, ch:2 * ch],
                    midx_reg, O1, rs1,
                    (t == 0, t == n_static - 1))

            # dynamic items
            for t in range(n_static, n_items):
                isl = nc.snap(((nchunk + cores - 1 - t) - c_reg) >> 3,
                              donate=True, min_val=0, max_val=1)
                blk = nc.snap(t - nchunk + c_reg + isl * (nchunk - c_reg),
                              donate=True, min_val=0, max_val=nchunk - 1)
                p_d = nc.snap(c_reg + isl * (nchunk - 1 - 2 * c_reg),
                              donate=True, min_val=0, max_val=nchunk - 1)
                dd = nc.snap(blk - p_d + 16, donate=True,
                             min_val=1, max_val=16)
                midx = nc.snap(dd >> 4, donate=True, min_val=0, max_val=1)

                qst = dynp.tile([128, ch], BF16, tag="qst")
                nc.vector.tensor_copy(qst[:], qT_bf[:, ds(isl * ch, ch)])
                kst = dynp.tile([128, 1, ch], BF16, tag="kst")
                nc.vector.tensor_copy(kst[:], kT_all[:, ds(blk, 1), :])
                vst = dynp.tile([128, 1, sub, dh], BF16, tag="vst")
                nc.vector.tensor_copy(vst[:], v_all[:, ds(blk, 1), :, :])

                o_part = misc_ps.tile([128, ch], F32, tag="mps")
                rs_part = misc_ps.tile([1, ch], F32, tag="mps")
                emit_item(
                    lambda u: kst[:, 0, 128 * u:128 * (u + 1)],
                    lambda u: vst[:, 0, u, :],
                    qst[:],
                    midx,
                    o_part, rs_part,
                    (True, True))
                nc.vector.tensor_add(
                    O_acc[:, ds(isl * ch, ch)],
                    O_acc[:, ds(isl * ch, ch)], o_part[:])
                nc.vector.tensor_add(
                    rs_acc[0:1, ds(isl * ch, ch)],
                    rs_acc[0:1, ds(isl * ch, ch)], rs_part[0:1, :])

            # ---------------- epilogue ----------------
            for h in range(2):
                Ot = epip.tile([128, ch], BF16, tag="Ot")
                rs_row = epip.tile([1, ch], F32, tag="rs_row")
                if h == 1:
                    nc.vector.tensor_add(Ot[:], O_acc[:, ch:2 * ch], O1[:])
                    nc.vector.tensor_add(rs_row[:], rs_acc[0:1, ch:2 * ch],
                                         rs1[0:1, :])
                else:
                    nc.vector.tensor_copy(Ot[:], O_acc[:, 0:ch])
                    nc.vector.tensor_copy(rs_row[:], rs_acc[0:1, 0:ch])
                rs_bf = epip.tile([1, ch], BF16, tag="rs_bf")
                nc.vector.tensor_copy(rs_bf[:], rs_row[:])
                for tt in range(it):
                    rsT = misc_ps.tile([128, 1], F32, tag="mps")
                    nc.tensor.matmul(
                        rsT[:],
                        lhsT=rs_row[0:1, 128 * tt:128 * (tt + 1)],
                        rhs=one_f[0:1, 0:1], start=True, stop=True)
                    rec = epip.tile([128, 1], F32, tag="rec")
                    nc.vector.reciprocal(rec[:], rsT[:])
                    for m in range(mh):
                        ops = misc_ps.tile([128, m_t], F32, tag="mps")
                        nc.tensor.matmul(
                            ops[:],
                            lhsT=Ot[:, 128 * tt:128 * (tt + 1)],
                            rhs=wout_bf[:, m * m_t:(m + 1) * m_t],
                            start=True, stop=False)
                        nc.tensor.matmul(
                            ops[:],
                            lhsT=rs_bf[0:1, 128 * tt:128 * (tt + 1)],
                            rhs=bout_bf[0:1, m * m_t:(m + 1) * m_t],
                            start=False, stop=True)
                        osb = outpp.tile([128, m_t], F32, tag="osb")
                        nc.vector.tensor_scalar_mul(osb[:], ops[:], rec[:, 0:1])
                        nc.sync.dma_start(
                            out[h * ch + 128 * tt: h * ch + 128 * (tt + 1),
                                m * m_t:(m + 1) * m_t],
                            osb[:])

    nc.compile()
    return nc


# ---------------- host side ----------------

_CACHED = {}


def _get_program(key, **kw):
    if key not in _CACHED:
        _CACHED[key] = build_program(**kw)
    return _CACHED[key]


def shard_inputs(x, w_qkv, b_qkv, w_out, b_out, cores=8):
    n = x.shape[0]
    nchunk = 2 * cores
    ch = n // nchunk
    in_maps = []
    for c in range(cores):
        xs = np.concatenate(
            [x[ch * c: ch * (c + 1)],
             x[ch * (nchunk - 1 - c): ch * (nchunk - c)]], axis=0)
        in_maps.append({
            "xT": np.ascontiguousarray(xs.T).astype(np.float32),
            "w_qkv": np.ascontiguousarray(w_qkv).astype(np.float32),
            "b_qkv": np.ascontiguousarray(
                b_qkv).reshape(1, -1).astype(np.float32),
            "w_out": np.ascontiguousarray(w_out).astype(np.float32),
            "b_out": np.ascontiguousarray(
                b_out).reshape(1, -1).astype(np.float32),
        })
    return in_maps


def unshard_output(results, n, d_out, cores=8):
    nchunk = 2 * cores
    ch = n // nchunk
    out = np.empty((n, d_out), dtype=np.float32)
    for c in range(cores):
        o = results[c]["out"]
        out[ch * c: ch * (c + 1)] = o[:ch]
        out[ch * (nchunk - 1 - c): ch * (nchunk - c)] = o[ch:]
    return out


def kernel(x, w_qkv, b_qkv, w_out, b_out):
    from concourse.bass_utils import run_bass_kernel_spmd

    x = np.asarray(x)
    w_qkv = np.asarray(w_qkv)
    b_qkv = np.asarray(b_qkv)
    w_out = np.asarray(w_out)
    b_out = np.asarray(b_out)
    cores = 8
    n, d_in = x.shape
    d_out = w_out.shape[1]
    dh = w_out.shape[0]
    nc = _get_program(
        (cores, n, d_in, d_out, dh),
        cores=cores, n=n, d_in=d_in, d_out=d_out, dh=dh)
    in_maps = shard_inputs(x, w_qkv, b_qkv, w_out, b_out, cores)
    res = run_bass_kernel_spmd(nc, in_maps, core_ids=list(range(cores)))
    return unshard_output(res.results, n, d_out, cores)


# revision 13
# speedup vs baseline: 1.1677x; 1.1268x over previous
"""Distributed causal attention kernel for one TRN2 chip (8 NeuronCores).

Problem: out = (softmax_causal((xWq)(xWk)^T / sqrt(dh)) (xWv)) Wout + b
  N=8192, D_IN=1024, D_HEAD=128, D_OUT=1024, fp32 I/O (bf16 compute).

Sharding (zig-zag for causal load balance): the sequence is split into
16 chunks of 512 rows; core c owns chunks c and 15-c, so every core has
the same causal attention area (17 blocks of 512x512).  Q stays local,
K/V shards are computed locally and AllGather'ed (bf16).

Layout: scores are computed transposed, St[j, i] = K Q^T, so that the
softmax-weighted PV matmul needs no transposes: O^T[dh, i] = V^T P^T via
lhsT = V (natural), rhs = exp(St).  Softmax skips the max-subtraction
(scores are ~N(0,1), |s| < ~7) and defers normalization: the row-sum is
accumulated with a ones-vector matmul, and the division happens after
the output projection (bias folded in exactly via a rank-1 matmul of
rowsum x b_out before the division).

SPMD uniformity: all cores run one program.  Of the 17 causal work
items per core, 9 are statically identical across cores; the remaining
8 select their (q-half, kv-block, mask) via DVE registers derived from
partition_id and dynamic `ds()` slices, with PV partials accumulated
into an SBUF accumulator by the vector engine.
"""

import os
import sys

import numpy as np

if "/opt/trn_rl_repo" not in sys.path:
    sys.path.insert(0, "/opt/trn_rl_repo")

import concourse.bass as bass
import concourse.mybir as mybir
import concourse.tile as tile
from concourse import bacc
from concourse.bass import ds

F32 = mybir.dt.float32
BF16 = mybir.dt.bfloat16
AF = mybir.ActivationFunctionType
ALU = mybir.AluOpType


def build_program(cores=8, n=8192, d_in=1024, d_out=1024, dh=128,
                  enable_asserts=False):
    nchunk = 2 * cores            # zig-zag chunks
    ch = n // nchunk              # rows per chunk (512)
    r = 2 * ch                    # rows per core (1024)
    kd = d_in // 128              # contraction chunks for projections
    sub = ch // 128               # 128-row sub-chunks per kv block
    it = ch // 128                # 128-row i-tiles per half
    scale = float(dh) ** -0.5
    sw = sub * ch                 # score tile width (free elems per item)
    mo = n // d_in if d_out >= 512 else 1  # unused; keep simple
    m_t = 512 if d_out >= 512 else d_out   # out-proj moving width
    mh = d_out // m_t

    nc = bacc.Bacc("TRN2", target_bir_lowering=False, debug=False,
                   num_devices=cores, enable_asserts=enable_asserts)

    xT = nc.dram_tensor("xT", [d_in, r], F32, kind="ExternalInput")
    w_qkv = nc.dram_tensor("w_qkv", [d_in, 3 * dh], F32, kind="ExternalInput")
    b_qkv = nc.dram_tensor("b_qkv", [1, 3 * dh], F32, kind="ExternalInput")
    w_out = nc.dram_tensor("w_out", [dh, d_out], F32, kind="ExternalInput")
    b_out = nc.dram_tensor("b_out", [1, d_out], F32, kind="ExternalInput")
    out = nc.dram_tensor("out", [r, d_out], F32, kind="ExternalOutput")

    with tile.TileContext(nc) as tc:
        with (
            tc.tile_pool(name="dram", bufs=1, space="DRAM") as dram,
            tc.tile_pool(name="consts", bufs=1) as consts,
            tc.tile_pool(name="params", bufs=1) as params,
            tc.tile_pool(name="qkv", bufs=1) as qkvp,
            tc.tile_pool(name="gath", bufs=1) as gath,
            tc.tile_pool(name="accs", bufs=1) as accs,
            tc.tile_pool(name="stage", bufs=3) as stagep,
            tc.tile_pool(name="exps", bufs=4) as exps,
            tc.tile_pool(name="dyn", bufs=3) as dynp,
            tc.tile_pool(name="epi", bufs=2) as epip,
            tc.tile_pool(name="outp", bufs=3) as outpp,
            tc.tile_pool(name="st_ps", bufs=2, space="PSUM") as st_ps,
            tc.tile_pool(name="o1_ps", bufs=1, space="PSUM") as o1_ps,
            tc.tile_pool(name="rs1_ps", bufs=1, space="PSUM") as rs1_ps,
            tc.tile_pool(name="misc_ps", bufs=2, space="PSUM") as misc_ps,
        ):
            # ---------------- constants ----------------
            ones_col = consts.tile([128, 1], BF16, tag="ones_col")
            nc.gpsimd.memset(ones_col[:], 1.0)
            ones_row = consts.tile([1, max(ch, 128)], BF16, tag="ones_row")
            nc.gpsimd.memset(ones_row[:], 1.0)
            one_f = consts.tile([1, 1], F32, tag="one_f")
            nc.gpsimd.memset(one_f[:], 1.0)
            # mask strips: [:, 0:sw] = ones, [:, sw:2*sw] = causal triangles
            masks = consts.tile([128, 2 * sw], BF16, tag="masks")
            nc.gpsimd.memset(masks[:], 1.0)
            for u in range(sub):
                # visible (keep 1.0) iff i_local >= 128*u + j_local
                nc.gpsimd.affine_select(
                    out=masks[:, sw + u * ch: sw + (u + 1) * ch],
                    in_=masks[:, sw + u * ch: sw + (u + 1) * ch],
                    compare_op=ALU.is_ge,
                    fill=0.0,
                    base=-(128 * u),
                    pattern=[[1, ch]],
                    channel_multiplier=-1,
                )

            # ---------------- params: load + cast to bf16 ----------------
            from concourse.tile_rust import add_dep_helper

            wqkv_bf = params.tile([128, kd, 3 * dh], BF16, tag="wqkv_bf")
            for k in range(kd):
                st = stagep.tile([128, 3 * dh], F32, tag="stage_w")
                nc.sync.dma_start(st[:], w_qkv[128 * k:128 * (k + 1), :])
                nc.vector.tensor_copy(wqkv_bf[:, k, :], st[:])
            bqkv_bf = params.tile([1, 3 * dh], BF16, tag="bqkv_bf")
            st = stagep.tile([1, 3 * dh], F32, tag="stage_b")
            nc.sync.dma_start(st[:], b_qkv[:, :])
            nc.vector.tensor_copy(bqkv_bf[:], st[:])

            # ------- per-half: load x half, project k/v, bounce + gather --
            # k/v for local half h are computed first so the AllGather of
            # half h overlaps the rest of the projection work.
            xT_bf = params.tile([128, kd, r], BF16, tag="xT_bf")
            qT_bf = qkvp.tile([128, r], BF16, tag="qT_bf")
            kT_loc = qkvp.tile([128, r], BF16, tag="kT_loc")
            v_loc = qkvp.tile([128, r // 128, dh], BF16, tag="v_loc")
            rg = [list(range(cores))]
            cc_insts = []
            for h in range(2):
                for k in range(kd):
                    st = stagep.tile([128, ch], F32, tag="stage_x")
                    nc.sync.dma_start(
                        st[:], xT[128 * k:128 * (k + 1),
                                  h * ch:(h + 1) * ch])
                    nc.vector.tensor_copy(
                        xT_bf[:, k, h * ch:(h + 1) * ch], st[:])
                # kT half h
                ps = misc_ps.tile([128, ch], F32, tag="mps")
                for k in range(kd):
                    nc.tensor.matmul(
                        ps[:],
                        lhsT=wqkv_bf[:, k, dh:2 * dh],
                        rhs=xT_bf[:, k, h * ch:(h + 1) * ch],
                        start=(k == 0), stop=False)
                nc.tensor.matmul(
                    ps[:], lhsT=bqkv_bf[0:1, dh:2 * dh],
                    rhs=ones_row[0:1, 0:ch], start=False, stop=True)
                nc.vector.tensor_copy(kT_loc[:, h * ch:(h + 1) * ch], ps[:])
                # v tiles of half h
                for t in range(sub * h, sub * h + sub):
                    ps = misc_ps.tile([128, dh], F32, tag="mps")
                    for k in range(kd):
                        nc.tensor.matmul(
                            ps[:],
                            lhsT=xT_bf[:, k, 128 * t:128 * (t + 1)],
                            rhs=wqkv_bf[:, k, 2 * dh:3 * dh],
                            start=(k == 0), stop=False)
                    nc.tensor.matmul(
                        ps[:], lhsT=ones_row[0:1, 0:128],
                        rhs=bqkv_bf[0:1, 2 * dh:3 * dh],
                        start=False, stop=True)
                    nc.vector.tensor_copy(v_loc[:, t, :], ps[:])
                # bounce + all-gather half h
                kv_b = dram.tile([2 * dh, ch], BF16, tag=f"kv_bounce{h}")
                kv_g = nc.dram_tensor(f"kv_gath{h}", [cores * 2 * dh, ch],
                                      BF16, addr_space="Shared")
                nc.sync.dma_start(kv_b[0:dh, :],
                                  kT_loc[:, h * ch:(h + 1) * ch])
                nc.sync.dma_start(
                    kv_b[dh:2 * dh, :].rearrange("p (t d) -> p t d", t=sub),
                    v_loc[:, sub * h:sub * h + sub, :])
                cc = nc.gpsimd.collective_compute(
                    "AllGather", ALU.bypass, replica_groups=rg,
                    ins=[kv_b.opt()], outs=[kv_g.ap().opt()])
                cc_insts.append((cc, kv_g))
            # q^T (after bounces, overlaps the gathers)
            for h in range(2):
                ps = misc_ps.tile([128, ch], F32, tag="mps")
                for k in range(kd):
                    nc.tensor.matmul(
                        ps[:],
                        lhsT=wqkv_bf[:, k, 0:dh],
                        rhs=xT_bf[:, k, h * ch:(h + 1) * ch],
                        start=(k == 0), stop=False)
                nc.tensor.matmul(
                    ps[:], lhsT=bqkv_bf[0:1, 0:dh],
                    rhs=ones_row[0:1, 0:ch], start=False, stop=True)
                nc.vector.tensor_copy(qT_bf[:, h * ch:(h + 1) * ch], ps[:])

            # remaining params
            wout_bf = params.tile([dh, d_out], BF16, tag="wout_bf")
            st = stagep.tile([dh, d_out], F32, tag="stage_w")
            nc.sync.dma_start(st[:dh, :d_out], w_out[:, :])
            nc.vector.tensor_copy(wout_bf[:], st[:dh, :d_out])
            bout_bf = params.tile([1, d_out], BF16, tag="bout_bf")
            st = stagep.tile([1, d_out], F32, tag="stage_b2")
            nc.sync.dma_start(st[:], b_out[:, :])
            nc.vector.tensor_copy(bout_bf[:], st[:])

            # stage gathered kv into SBUF, in global block order.
            # half-h gather rows [256rr, 256rr+128) = core rr's kT half;
            # rows [256rr+128, 256rr+256): row 128+p col 128t+d = v half
            # tile t row p col d.
            kT_all = gath.tile([128, nchunk, ch], BF16, tag="kT_all")
            v_all = gath.tile([128, nchunk, sub, dh], BF16, tag="v_all")
            for b in range(nchunk):
                rr, hh = (b, 0) if b < cores else (nchunk - 1 - b, 1)
                cc, kv_g = cc_insts[hh]
                base = 2 * dh * rr
                d1 = nc.sync.dma_start(
                    kT_all[:, b, :], kv_g[base:base + 128, :])
                d2 = nc.sync.dma_start(
                    v_all[:, b, :, :],
                    kv_g[base + 128:base + 256, :]
                    .rearrange("p (u d) -> p u d", u=sub))
                add_dep_helper(d1.ins, cc.ins, sync=True,
                               reason="gather staging waits on collective")
                add_dep_helper(d2.ins, cc.ins, sync=True,
                               reason="gather staging waits on collective")

            # ---------------- attention ----------------
            # per-core registers (vector engine only)
            c_reg = nc.vector.partition_id()

            O_acc = accs.tile([128, 2 * ch], F32, tag="O_acc")
            rs_acc = accs.tile([1, 2 * ch], F32, tag="rs_acc")
            nc.gpsimd.memset(O_acc[:], 0.0)
            nc.gpsimd.memset(rs_acc[:], 0.0)

            O1 = o1_ps.tile([128, ch], F32, tag="O1")
            rs1 = rs1_ps.tile([1, ch], F32, tag="rs1")

            n_static = cores              # items 0..cores-1: half 1, block=t
            n_items = nchunk + 1

            gs = max(1, sub // 2)  # subchunks per St group (double-buffer)

            def emit_item(k_ap_fn, v_ap_fn, q_ap, midx_reg, o_ps, rs_ps,
                          o_startstop, mask_static=False):
                """k_ap_fn(u) -> [128,128] lhsT; v_ap_fn(u) -> [128,dh] lhsT."""
                o_start, o_stop = o_startstop
                for g in range(0, sub, gs):
                    us = range(g, min(sub, g + gs))
                    gw = len(us) * ch
                    stp = st_ps.tile([128, gs * ch], F32, tag="St")
                    for ui, u in enumerate(us):
                        nc.tensor.matmul(
                            stp[:, ui * ch:(ui + 1) * ch],
                            lhsT=k_ap_fn(u), rhs=q_ap,
                            start=True, stop=True)
                    ex = exps.tile([128, gs * ch], BF16, tag="ex")
                    nc.scalar.activation(ex[:, :gw], stp[:, :gw], AF.Exp,
                                         scale=scale)
                    if mask_static:
                        nc.vector.tensor_mul(
                            ex[:, :gw], ex[:, :gw],
                            masks[:, sw + g * ch: sw + g * ch + gw])
                    elif midx_reg is not None:
                        nc.vector.tensor_mul(
                            ex[:, :gw], ex[:, :gw],
                            masks[:, ds(midx_reg * sw + g * ch, gw)])
                    for ui, u in enumerate(us):
                        nc.tensor.matmul(
                            o_ps[:],
                            lhsT=v_ap_fn(u), rhs=ex[:, ui * ch:(ui + 1) * ch],
                            start=(o_start and u == 0),
                            stop=(o_stop and u == sub - 1))
                    for ui, u in enumerate(us):
                        nc.tensor.matmul(
                            rs_ps[0:1, :],
                            lhsT=ones_col[:, 0:1],
                            rhs=ex[:, ui * ch:(ui + 1) * ch],
                            start=(o_start and u == 0),
                            stop=(o_stop and u == sub - 1))

            # static items: half 1 (chunk nchunk-1-c), blocks 0..cores
            for t in range(n_static):
                b = t
                midx_reg = None
                emit_item(
                    lambda u, b=b: kT_all[:, b, 128 * u:128 * (u + 1)],
                    lambda u, b=b: v_all[:, b, u, :],
                    qT_bf[:, ch:2 * ch],
                    midx_reg, O1, rs1,
                    (t == 0, t == n_static - 1))

            # dynamic items
            for t in range(n_static, n_items):
                isl = nc.snap(((nchunk + cores - 1 - t) - c_reg) >> 3,
                              donate=True, min_val=0, max_val=1)
                blk = nc.snap(t - nchunk + c_reg + isl * (nchunk - c_reg),
                              donate=True, min_val=0, max_val=nchunk - 1)
                p_d = nc.snap(c_reg + isl * (nchunk - 1 - 2 * c_reg),
                              donate=True, min_val=0, max_val=nchunk - 1)
                dd = nc.snap(blk - p_d + 16, donate=True,
                             min_val=1, max_val=16)
                midx = nc.snap(dd >> 4, donate=True, min_val=0, max_val=1)

                qst = dynp.tile([128, ch], BF16, tag="qst")
                nc.vector.tensor_copy(qst[:], qT_bf[:, ds(isl * ch, ch)])
                kst = dynp.tile([128, 1, ch], BF16, tag="kst")
                nc.vector.tensor_copy(kst[:], kT_all[:, ds(blk, 1), :])
                vst = dynp.tile([128, 1, sub, dh], BF16, tag="vst")
                nc.vector.tensor_copy(vst[:], v_all[:, ds(blk, 1), :, :])

                o_part = misc_ps.tile([128, ch], F32, tag="mps")
                rs_part = misc_ps.tile([1, ch], F32, tag="mps")
                emit_item(
                    lambda u: kst[:, 0, 128 * u:128 * (u + 1)],
                    lambda u: vst[:, 0, u, :],
                    qst[:],
                    midx,
                    o_part, rs_part,
                    (True, True))
                nc.vector.tensor_add(
                    O_acc[:, ds(isl * ch, ch)],
                    O_acc[:, ds(isl * ch, ch)], o_part[:])
                nc.vector.tensor_add(
                    rs_acc[0:1, ds(isl * ch, ch)],
                    rs_acc[0:1, ds(isl * ch, ch)], rs_part[0:1, :])

            # ---------------- epilogue ----------------
            for h in range(2):
                Ot = epip.tile([128, ch], BF16, tag="Ot")
                rs_row = epip.tile([1, ch], F32, tag="rs_row")
                if h == 1:
                    nc.vector.tensor_add(Ot[:], O_acc[:, ch:2 * ch], O1[:])
                    nc.vector.tensor_add(rs_row[:], rs_acc[0:1, ch:2 * ch],
                                         rs1[0:1, :])
                else:
                    nc.vector.tensor_copy(Ot[:], O_acc[:, 0:ch])
                    nc.vector.tensor_copy(rs_row[:], rs_acc[0:1, 0:ch])
                rs_bf = epip.tile([1, ch], BF16, tag="rs_bf")
                nc.vector.tensor_copy(rs_bf[:], rs_row[:])
                for tt in range(it):
                    rsT = misc_ps.tile([128, 1], F32, tag="mps")
                    nc.tensor.matmul(
                        rsT[:],
                        lhsT=rs_row[0:1, 128 * tt:128 * (tt + 1)],
                        rhs=one_f[0:1, 0:1], start=True, stop=True)
                    rec = epip.tile([128, 1], F32, tag="rec")
                    nc.vector.reciprocal(rec[:], rsT[:])
                    for m in range(mh):
                        ops = misc_ps.tile([128, m_t], F32, tag="mps")
                        nc.tensor.matmul(
                            ops[:],
                            lhsT=Ot[:, 128 * tt:128 * (tt + 1)],
                            rhs=wout_bf[:, m * m_t:(m + 1) * m_t],
                            start=True, stop=False)
                        nc.tensor.matmul(
                            ops[:],
                            lhsT=rs_bf[0:1, 128 * tt:128 * (tt + 1)],
                            rhs=bout_bf[0:1, m * m_t:(m + 1) * m_t],
                            start=False, stop=True)
                        osb = outpp.tile([128, m_t], F32, tag="osb")
                        nc.vector.tensor_scalar_mul(osb[:], ops[:], rec[:, 0:1])
                        nc.sync.dma_start(
                            out[h * ch + 128 * tt: h * ch + 128 * (tt + 1),
                                m * m_t:(m + 1) * m_t],
                            osb[:])

    nc.compile()
    return nc


# ---------------- host side ----------------

_CACHED = {}


def _get_program(key, **kw):
    if key not in _CACHED:
        _CACHED[key] = build_program(**kw)
    return _CACHED[key]


def shard_inputs(x, w_qkv, b_qkv, w_out, b_out, cores=8):
    n = x.shape[0]
    nchunk = 2 * cores
    ch = n // nchunk
    in_maps = []
    for c in range(cores):
        xs = np.concatenate(
            [x[ch * c: ch * (c + 1)],
             x[ch * (nchunk - 1 - c): ch * (nchunk - c)]], axis=0)
        in_maps.append({
            "xT": np.ascontiguousarray(xs.T).astype(np.float32),
            "w_qkv": np.ascontiguousarray(w_qkv).astype(np.float32),
            "b_qkv": np.ascontiguousarray(
                b_qkv).reshape(1, -1).astype(np.float32),
            "w_out": np.ascontiguousarray(w_out).astype(np.float32),
            "b_out": np.ascontiguousarray(
                b_out).reshape(1, -1).astype(np.float32),
        })
    return in_maps


def unshard_output(results, n, d_out, cores=8):
    nchunk = 2 * cores
    ch = n // nchunk
    out = np.empty((n, d_out), dtype=np.float32)
    for c in range(cores):
        o = results[c]["out"]
        out[ch * c: ch * (c + 1)] = o[:ch]
        out[ch * (nchunk - 1 - c): ch * (nchunk - c)] = o[ch:]
    return out


def kernel(x, w_qkv, b_qkv, w_out, b_out):
    from concourse.bass_utils import run_bass_kernel_spmd

    x = np.asarray(x)
    w_qkv = np.asarray(w_qkv)
    b_qkv = np.asarray(b_qkv)
    w_out = np.asarray(w_out)
    b_out = np.asarray(b_out)
    cores = 8
    n, d_in = x.shape
    d_out = w_out.shape[1]
    dh = w_out.shape[0]
    nc = _get_program(
        (cores, n, d_in, d_out, dh),
        cores=cores, n=n, d_in=d_in, d_out=d_out, dh=dh)
    in_maps = shard_inputs(x, w_qkv, b_qkv, w_out, b_out, cores)
    res = run_bass_kernel_spmd(nc, in_maps, core_ids=list(range(cores)))
    return unshard_output(res.results, n, d_out, cores)
